# revision 1
# baseline (speedup 1.0000x reference)
"""DiagonalAttention Trainium2 kernel (Bass/Tile), data-parallel over batch on 8 cores.

Reference computation (per batch b):
    r1 = relu(x1 @ W.T) * diagonal          [L, H]
    r2 = relu(x2 @ W.T)                     [L, H]
    s  = r1 @ r2.T + (1-mask)*NEG           [L, L]
    out = softmax(s, -1) @ x2               [L, D]

Device strategy per core (2 batches/core):
  - host: transpose x1/x2 to [D, L], tf32-round them + W (fp32r matmuls run at
    1 cyc/row on the PE), cast x2 to bf16 for the output matmul.
  - proj (fp32r): rT[h, l] accumulated over d-chunks, relu on ScalarE -> fp32r.
  - scores (fp32r): psum[i=128, j=2048], mask row added via a K=1 bf16 starter
    matmul; ScalarE copies scores to SBUF (single-reader psum), VectorE row-max,
    ScalarE exp(s-max) -> bf16 E with fused row-sum (accum_out).
  - E transposed 128x128 on the PE (bf16), bmm3 = ET.T @ x2_bf16 accumulated in
    psum, scaled by 1/z on VectorE during psum->SBUF copy, DMA out.

Every PE/ACT/DVE instruction is limited to ONE semaphore wait by walrus codegen;
tiny "absorber" ops (PE corner-transposes into a persistent psum tile, DVE corner
reads/memsets, ACT bias bounce) pre-observe semaphores so no instruction ever
needs two.
"""
import numpy as np

B, L, D, H = 16, 2048, 1024, 1024
NCORES = 8
B_LOC = B // NCORES
NEG = -10000.0

ND = D // 128   # d chunks
NH = H // 128   # h chunks
NI = L // 128   # i chunks per batch
SW = 256        # proj slab width (moving-dim of fp32r matmuls)
NS = L // SW    # slabs per batch
IPS = SW // 128  # i-chunks per slab
JW = 512        # bmm2 moving width
NJ = L // JW    # j chunks in bmm2
NJ3 = L // 128  # j chunks in bmm3 (stationary ET tiles)

_PROG = None


def tf32_round(x):
    xi = np.ascontiguousarray(x, dtype=np.float32).view(np.uint32)
    return ((xi + 0x1000) & 0xFFFFE000).view(np.float32)


def _build_program(use_mask_starter=True, separate_w1=False, sw=SW):
    import concourse.bass as bass
    import concourse.tile as tile
    from concourse import mybir
    from concourse.bass import _add_dep_helper
    from concourse.masks import make_identity

    def order(first, then):
        _add_dep_helper(then.ins, first.ins, sync=False, reason="order")
        return then

    dt = mybir.dt
    nc = bass.Bass("TRN2", target_bir_lowering=False, debug=False)

    x1T = nc.dram_tensor("x1T", [B_LOC, D, L], dt.float32r, kind="ExternalInput").ap()
    x2T = nc.dram_tensor("x2T", [B_LOC, D, L], dt.float32r, kind="ExternalInput").ap()
    WT = nc.dram_tensor("WT", [D, H], dt.float32r, kind="ExternalInput").ap()
    W1T = nc.dram_tensor("W1T", [D, H], dt.float32r, kind="ExternalInput").ap()
    x2n = nc.dram_tensor("x2n", [B_LOC, L, D], dt.bfloat16, kind="ExternalInput").ap()
    ns = L // sw
    ips = sw // 128
    mrow = nc.dram_tensor("mrow", [B_LOC, 1, L], dt.bfloat16, kind="ExternalInput").ap()
    out = nc.dram_tensor("out", [B_LOC, L, D], dt.float32, kind="ExternalOutput").ap()
    dscr_list = [nc.dram_tensor(f"dscr{k}", [1, 4], dt.float32r).ap()
                 for k in range(256)]
    scr_idx = [0]

    with tile.TileContext(nc) as tc:
        with (
            tc.tile_pool(name="const", bufs=1) as cpool,
            tc.tile_pool(name="big", bufs=1) as bigp,
            tc.tile_pool(name="slab", bufs=2) as slabp,
            tc.tile_pool(name="work", bufs=1) as workp,
            tc.tile_pool(name="work2", bufs=2) as work2p,
            tc.tile_pool(name="outp", bufs=4) as outp,
            tc.tile_pool(name="ps_s", bufs=1, space="PSUM") as ps_s,
            tc.tile_pool(name="ps_p", bufs=1, space="PSUM") as ps_p,
            tc.tile_pool(name="ps_sm", bufs=2, space="PSUM") as ps_sm,
            tc.tile_pool(name="ps_ab", bufs=1, space="PSUM") as ps_abp,
        ):
            identbf = cpool.tile([128, 128], dt.bfloat16, tag="identbf")
            make_identity(nc, identbf[:])
            identr = cpool.tile([32, 32], dt.float32r, tag="identr")
            make_identity(nc, identr[:])
            ones_bf = cpool.tile([1, 128], dt.bfloat16, tag="ones_bf")
            nc.vector.memset(ones_bf[:], 1.0)
            dve_ab = cpool.tile([1, 8], dt.float32, tag="dve_ab")
            sbscr = cpool.tile([1, 1024], dt.float32r, tag="sbscr")
            act_ab = cpool.tile([1, 8], dt.float32, tag="act_ab")
            echo = cpool.tile([32, 32], dt.float32r, tag="echo")

            ps_ab_t = ps_abp.tile([32, 64], dt.bfloat16, tag="ps_ab")
            ps_ab = ps_ab_t[:, 0:64].bitcast(dt.float32r)
            # warmup: PE observes GPSIMD (identity producer)
            nc.tensor.transpose(ps_ab, identr[:], identr[:])

            def absorb_r(corner_ap):
                # PE pre-observes the semaphore guarding corner_ap (1 wait)
                return nc.tensor.transpose(ps_ab, corner_ap, identr[:])

            def absorb_bf(corner_ap):
                return nc.tensor.transpose(ps_ab_t[:, 0:32], corner_ap,
                                           identbf[0:32, 0:32])

            def sp_sync(dep_inst):
                # SP observes dep_inst's engine tick via a write-once scratch
                d = nc.sync.dma_start(dscr_list[scr_idx[0]][0:1, 0:1],
                                      WT[0:1, 0:1])
                scr_idx[0] += 1
                _add_dep_helper(d.ins, dep_inst.ins, sync=True,
                                reason="sp sync absorb")
                return d

            def sp_absorb(corner_ap):
                # SP pre-observes the DMA semaphore guarding corner_ap (1 wait)
                v = corner_ap.bitcast(dt.float32r)
                n = v.free_size()
                nc.sync.dma_start(dscr_list[scr_idx[0]][0:1, 0:n], v)
                scr_idx[0] += 1

            # resident weights
            wt = bigp.tile([128, ND, H], dt.float32r, tag="wt")
            for dc in range(ND):
                nc.sync.dma_start(wt[:, dc, :],
                                  WT.rearrange("(c p) h -> p c h", p=128)[:, dc, :])
            for dc in range(ND):
                absorb_r(wt[0:32, dc, 0:32])
            if separate_w1:
                w1t = bigp.tile([128, ND, H], dt.float32r, tag="w1t")
                for dc in range(ND):
                    nc.sync.dma_start(
                        w1t[:, dc, :],
                        W1T.rearrange("(c p) h -> p c h", p=128)[:, dc, :])
                for dc in range(ND):
                    absorb_r(w1t[0:32, dc, 0:32])
            else:
                # diagonal == 1: W1 is W, share the resident tile (the W1T dram
                # input is still bound; a token DMA keeps it referenced)
                w1t = wt
                w1tok = cpool.tile([1, 8], dt.float32r, tag="w1tok")
                nc.sync.dma_start(w1tok[:], W1T[0:1, 0:8])

            # resident per-batch tensors
            r2T = bigp.tile([128, NH, L], dt.float32r, tag="r2T")
            t_x2n = bigp.tile([128, NJ3, D], dt.bfloat16, tag="t_x2n")
            t_m = workp.tile([1, L], dt.bfloat16, tag="t_m")
            sco = workp.tile([128, L], dt.float32, tag="sco")
            te = workp.tile([128, L], dt.bfloat16, tag="te")
            tET = workp.tile([128, NJ3, 128], dt.bfloat16, tag="tET")

            prev_relu_corner = [None]
            first_chunk = [True]
            slab_alloc_count = [0]
            slot_last_mm = [None, None]
            prev_out = [None]
            last_bmm3_mm = [None]
            last_starter = [None]
            for b in range(B_LOC):
                # ---- batch loads ----
                if last_bmm3_mm[0] is not None:
                    sp_sync(last_bmm3_mm[0])
                for jc in range(NJ3):
                    nc.sync.dma_start(
                        t_x2n[:, jc, :],
                        x2n[b].rearrange("(c p) d -> p c d", p=128)[:, jc, :])
                for jc in range(NJ3):
                    absorb_bf(t_x2n[0:32, jc, 0:32])
                if last_starter[0] is not None:
                    sp_sync(last_starter[0])
                nc.sync.dma_start(t_m[:], mrow[b])

                # ---- proj2: r2T = relu(W @ x2T) over all slabs ----
                for s in range(ns):
                    slot = slab_alloc_count[0] % 2
                    slab_alloc_count[0] += 1
                    if slot_last_mm[slot] is not None:
                        sp_sync(slot_last_mm[slot])
                    xs = slabp.tile([128, ND, sw], dt.float32r, tag="xslab")
                    for dc in range(ND):
                        nc.sync.dma_start(
                            xs[:, dc, :],
                            x2T[b].rearrange("(c p) l -> p c l", p=128)[
                                :, dc, s * sw:(s + 1) * sw],
                        )
                    for dc in range(ND):
                        absorb_r(xs[0:32, dc, 0:32])
                    for hc in range(NH):
                        if prev_relu_corner[0] is not None:
                            absorb_r(prev_relu_corner[0])
                        psp = ps_p.tile([128, sw], dt.float32, tag="psp")
                        for dc in range(ND):
                            mm = nc.tensor.matmul(
                                psp[:], wt[:, dc, hc * 128:(hc + 1) * 128],
                                xs[:, dc, :],
                                start=(dc == 0), stop=(dc == ND - 1),
                            )
                            slot_last_mm[slot] = mm
                        nc.scalar.activation(
                            r2T[:, hc, s * sw:(s + 1) * sw], psp[:],
                            mybir.ActivationFunctionType.Relu)
                        prev_relu_corner[0] = r2T[0:32, hc, s * sw:s * sw + 32]

                # ---- proj1 + attention, slab by slab ----
                for s in range(ns):
                    slot = slab_alloc_count[0] % 2
                    slab_alloc_count[0] += 1
                    if slot_last_mm[slot] is not None:
                        sp_sync(slot_last_mm[slot])
                    xs = slabp.tile([128, ND, sw], dt.float32r, tag="xslab")
                    for dc in range(ND):
                        nc.sync.dma_start(
                            xs[:, dc, :],
                            x1T[b].rearrange("(c p) l -> p c l", p=128)[
                                :, dc, s * sw:(s + 1) * sw],
                        )
                    for dc in range(ND):
                        absorb_r(xs[0:32, dc, 0:32])
                    r1s = slabp.tile([128, NH, sw], dt.float32r, tag="r1slab")
                    for hc in range(NH):
                        if prev_relu_corner[0] is not None:
                            absorb_r(prev_relu_corner[0])
                        psp = ps_p.tile([128, sw], dt.float32, tag="psp")
                        for dc in range(ND):
                            mm = nc.tensor.matmul(
                                psp[:], w1t[:, dc, hc * 128:(hc + 1) * 128],
                                xs[:, dc, :],
                                start=(dc == 0), stop=(dc == ND - 1),
                            )
                            slot_last_mm[slot] = mm
                        nc.scalar.activation(
                            r1s[:, hc, :], psp[:],
                            mybir.ActivationFunctionType.Relu)
                        prev_relu_corner[0] = r1s[0:32, hc, 0:32]

                    for il in range(ips):
                        ic = s * ips + il
                        isl = slice(il * 128, (il + 1) * 128)
                        # PE pre-observes relu of this slab + sco-copy(i-1)
                        pe_last = absorb_r(
                            r1s[0:32, NH - 1, il * 128:il * 128 + 32])
                        if not first_chunk[0]:
                            pe_last = order(pe_last, absorb_r(echo[:]))
                        pss = ps_s.tile([128, L], dt.float32, tag="pss")
                        # first-writer corner absorber takes the slot-recycle
                        pe_last = order(pe_last, nc.tensor.transpose(
                            pss[0:32, 0:32].bitcast(dt.float32r), identr[:],
                            identr[:]))
                        if use_mask_starter:
                            for jc in range(NJ):
                                pe_last = order(pe_last, nc.tensor.matmul(
                                    pss[:, jc * JW:(jc + 1) * JW], ones_bf[:],
                                    t_m[:, jc * JW:(jc + 1) * JW],
                                    start=True, stop=False,
                                    skip_group_check=True))
                                last_starter[0] = pe_last
                        for hc in range(NH):
                            for jc in range(NJ):
                                pe_last = order(pe_last, nc.tensor.matmul(
                                    pss[:, jc * JW:(jc + 1) * JW],
                                    r1s[:, hc, isl],
                                    r2T[:, hc, jc * JW:(jc + 1) * JW],
                                    start=(not use_mask_starter and hc == 0),
                                    stop=(hc == NH - 1),
                                    skip_group_check=True))
                        # ACT probes: observe own tail (te) then PE (pss)
                        act_last = None
                        if not first_chunk[0]:
                            act_last = nc.scalar.copy(act_ab[0:1, 0:1],
                                                      te[0:1, 0:1])
                        a = nc.scalar.copy(act_ab[0:1, 1:2], pss[0:1, 0:1])
                        act_last = order(act_last, a) if act_last else a
                        act_last = order(act_last,
                                         nc.scalar.copy(sco[:], pss[:]))
                        # fp32r echo of the copy tick for the next chunk's PE
                        act_last = order(act_last,
                                         nc.scalar.copy(echo[:],
                                                        sco[0:32, 0:32]))
                        tneg = work2p.tile([128, 1], dt.float32, tag="tneg")
                        dve_last = nc.vector.tensor_reduce(
                            tneg[:], sco[:], axis=mybir.AxisListType.X,
                            op=mybir.AluOpType.max, negate=True)
                        tnega = work2p.tile([128, 1], dt.float32, tag="tnega")
                        act_last = order(act_last,
                                         nc.scalar.copy(tnega[:], tneg[:]))
                        tz = work2p.tile([128, 1], dt.float32, tag="tz")
                        act_last = order(act_last, nc.scalar.activation(
                            te[:], sco[:], mybir.ActivationFunctionType.Exp,
                            bias=tnega[:], scale=1.0, accum_out=tz[:]))
                        # transpose E
                        for jc in range(NJ3):
                            pst = ps_sm.tile([128, 128], dt.bfloat16, tag="psm")
                            pe_last = order(pe_last, nc.tensor.transpose(
                                pst[:], te[:, jc * 128:(jc + 1) * 128],
                                identbf[:]))
                            dve_last = order(dve_last, nc.vector.tensor_copy(
                                tET[:, jc, :], pst[:]))
                        # DVE probe1: observe own tail (last ET copy)
                        dve_last = order(dve_last, nc.vector.tensor_copy(
                            dve_ab[0:1, 0:1], tET[0:1, NJ3 - 1, 0:1]))
                        tzi = work2p.tile([128, 1], dt.float32, tag="tzi")
                        dve_last = order(dve_last,
                                         nc.vector.reciprocal(tzi[:], tz[:]))
                        # bmm3 in two d-halves (1 psum bank each)
                        for dh in range(2):
                            pso = ps_sm.tile([128, 512], dt.float32, tag="psm")
                            dsl = slice(dh * 512, (dh + 1) * 512)
                            for jc in range(NJ3):
                                pe_last = order(pe_last, nc.tensor.matmul(
                                    pso[:], tET[:, jc, :],
                                    t_x2n[:, jc, dsl],
                                    start=(jc == 0), stop=(jc == NJ3 - 1)))
                                last_bmm3_mm[0] = pe_last
                            tout = outp.tile([128, 512], dt.float32,
                                             tag="tout")
                            # DVE probe2 (PE), probe3 (output-DMA WAR)
                            dve_last = order(dve_last, nc.vector.tensor_copy(
                                dve_ab[0:1, 1:2], pso[0:1, 0:1]))
                            dve_last = order(dve_last,
                                             nc.vector.memset(tout[0:1, 0:1],
                                                              0.0))
                            dve_last = order(dve_last,
                                             nc.vector.tensor_scalar_mul(
                                                 tout[:], pso[:], tzi[:]))
                            if prev_out[0] is not None:
                                sp_absorb(prev_out[0])
                            nc.sync.dma_start(
                                out[b, ic * 128:(ic + 1) * 128, dsl], tout[:])
                            prev_out[0] = out[b, ic * 128:ic * 128 + 1, dsl][:, 0:2]
                        first_chunk[0] = False
    return nc


def _prepare_inputs(x1, x2, x2_mask, W, diagonal):
    import ml_dtypes
    x1 = np.ascontiguousarray(x1, dtype=np.float32)
    x2 = np.ascontiguousarray(x2, dtype=np.float32)
    W = np.ascontiguousarray(W, dtype=np.float32)
    diagonal = np.asarray(diagonal, dtype=np.float32)
    mask = np.asarray(x2_mask).astype(np.float32)

    assert np.all(diagonal > 0), "kernel fast path requires diagonal > 0"
    WT = tf32_round(W.T.copy())
    shared_w = bool(np.all(diagonal == 1.0))
    if shared_w:
        W1T = WT
    else:
        W1T = tf32_round((W * diagonal[:, None]).T.copy())

    x1T = tf32_round(np.ascontiguousarray(x1.transpose(0, 2, 1)))
    x2T = tf32_round(np.ascontiguousarray(x2.transpose(0, 2, 1)))
    x2nb = x2.astype(ml_dtypes.bfloat16)
    mrow = ((1.0 - mask) * NEG)[:, None, :].astype(ml_dtypes.bfloat16)

    global _PROG
    if _PROG is None:
        _PROG = _build_program(separate_w1=not shared_w,
                               sw=SW if shared_w else 256)
    in_maps = []
    for c in range(NCORES):
        bs = slice(c * B_LOC, (c + 1) * B_LOC)
        in_maps.append({
            "x1T": x1T[bs],
            "x2T": x2T[bs],
            "WT": WT,
            "W1T": W1T,
            "x2n": x2nb[bs],
            "mrow": mrow[bs],
        })
    return in_maps


def _get_program():
    global _PROG
    if _PROG is None:
        _PROG = _build_program()
    return _PROG


def run(inputs, trace=False):
    """Run and return (output, BassKernelResults)."""
    from concourse.bass_utils import run_bass_kernel_spmd
    nc = _get_program()
    in_maps = _prepare_inputs(**inputs)
    res = run_bass_kernel_spmd(nc, in_maps, core_ids=list(range(NCORES)),
                               trace=trace)
    outs = [res.results[c]["out"] for c in range(NCORES)]
    full = np.concatenate(outs, axis=0).astype(np.float32)
    return full, res


# ---------------------------------------------------------------------------
# Shipping path: data-parallel jax/XLA over the 8 NeuronCores via shard_map.
# (The Bass path above compiles to IR that the current walrus rejects due to
# its 1-sync-wait-per-instruction limit on DMA queue gating; see notes.)
_JFN = None


def _jax_kernel():
    global _JFN
    if _JFN is not None:
        return _JFN
    import jax
    import jax.numpy as jnp
    from jax.sharding import Mesh, PartitionSpec as P
    from jax.experimental.shard_map import shard_map

    devices = jax.devices()[:NCORES]
    mesh = Mesh(np.asarray(devices), ("b",))

    def body(x1, x2, m, W, diag):
        r1 = jax.nn.relu(jnp.einsum("bld,hd->blh", x1, W)) * diag
        r2 = jax.nn.relu(jnp.einsum("bld,hd->blh", x2, W))
        s = jnp.einsum("bih,bjh->bij", r1, r2)
        s = s + m[:, None, :]
        a = jax.nn.softmax(s, axis=-1)
        return jnp.einsum("bij,bjd->bid", a, x2)

    fn = jax.jit(shard_map(
        body, mesh=mesh,
        in_specs=(P("b"), P("b"), P("b"), P(), P()),
        out_specs=P("b"), check_rep=False))
    _JFN = fn
    return fn


def kernel(**inputs) -> np.ndarray:
    import jax
    x1 = np.ascontiguousarray(inputs["x1"], dtype=np.float32)
    x2 = np.ascontiguousarray(inputs["x2"], dtype=np.float32)
    W = np.ascontiguousarray(inputs["W"], dtype=np.float32)
    diag = np.asarray(inputs["diagonal"], dtype=np.float32)
    m = ((1.0 - np.asarray(inputs["x2_mask"]).astype(np.float32)) * NEG)
    fn = _jax_kernel()
    out = fn(x1, x2, m.astype(np.float32), W, diag)
    return np.asarray(jax.device_get(out)).astype(np.float32)



# revision 6
# speedup vs baseline: 1.1069x; 1.1069x over previous
"""DiagonalAttention Trainium2 kernel (Bass/Tile), data-parallel over batch.

Reference computation per batch b (L=2048, D=H=1024):
    r1 = relu(x1 @ W.T) * diag        [L, H]   (diag>0 folded into W1)
    r2 = relu(x2 @ W.T)               [L, H]
    s  = r1 @ r2.T + (1-mask)*NEG     [L, L]
    out = softmax(s, -1) @ x2         [L, D]

Device strategy per core (B_LOC=2 batches):
  - host: transpose x1/x2 to [D, L] fp16, W/W1 -> WT fp16, x2 bf16 copy for
    the output matmul, mask row bf16.
  - proj (fp16 matmuls, fp32 psum): rT[h, l] accumulated over d-chunks,
    relu on ScalarE -> fp16.
  - scores (fp16): psum[i=128, j=2048]; mask row added via K=1 bf16 starter
    matmuls; ScalarE copies scores to SBUF, VectorE row-max, ScalarE
    exp(s-max) -> bf16 E with fused row-sum (accum_out).
  - E transposed 128x128 on the PE (bf16); bmm3 = ET.T @ x2_bf16 in psum,
    scaled by 1/z on VectorE during psum->SBUF copy, DMA out.
  - PE stream software-pipelined one i-chunk: transposes+bmm3 of chunk i
    are emitted after the scores matmuls of chunk i+1, so the PE never
    stalls on the softmax chain.

This container's walrus allows ONE sync wait per instruction; the
legalization pass below splits multi-wait instructions (absorber DMAs on
the same ring for DMAs — HWDGE rings evaluate descriptor waits in FIFO
order — and NoOps on the same engine otherwise).
"""
import copy
import numpy as np

B, L, D, H = 16, 2048, 1024, 1024
NCORES = 8
B_LOC = B // NCORES
NEG = -10000.0

ND = D // 128    # d chunks (contraction of proj)
NH = H // 128    # h chunks
NI = L // 128    # i chunks per batch
SW = 512         # proj slab width (moving dim)
NS = L // SW     # slabs per batch
IPS = SW // 128  # i-chunks per slab
JW = 512         # scores moving width
NJ = L // JW     # j chunks in scores
NJ3 = L // 128   # j chunks in bmm3 (stationary ET tiles)


# ---------------------------------------------------------------------------
# Wait-count legalization


def _make_wait_scratch(nc):
    """Scratch DRAM + token DMA (call inside the TileContext); the token
    gives a fully-lowered physical-AP DMACopy to clone as absorber."""
    from concourse import mybir

    src = nc.dram_tensor("legal_src", [1, 16], mybir.dt.float32, kind="Internal")
    dst = nc.dram_tensor("legal_dst", [1, 16], mybir.dt.float32, kind="Internal")
    tok = nc.sync.dma_start(dst.ap()[0:1, 0:1], src.ap()[0:1, 0:1])
    return tok.ins


def _legalize_waits(nc, template_inst, max_waits=1):
    """Split every instruction with more than max_waits sync waits.

    DMACopy: insert tiny absorber DMAs on the same queue (ring-FIFO makes
    the carried waits gate the real DMA).  Engine instructions: insert
    NoOps on the same engine (engines dispatch strictly in order; a
    waiting NoOp stalls everything behind it).
    """
    from concourse import mybir

    sem = nc.alloc_semaphore("legal_junk")
    junk = mybir.SyncUpdate(
        sync_type="semaphore", id=getattr(sem, "num", None),
        update_mode="sem-add-imm", update_value=16,
        ant_name=getattr(sem, "name", "legal_junk"))
    for fn in nc.m.functions:
        for blk in fn.blocks:
            out = []
            for inst in blk.instructions:
                si = getattr(inst, "sync_info", None)
                if si is None or si.on_wait is None or len(si.on_wait) <= max_waits:
                    out.append(inst)
                    continue
                waits = list(si.on_wait)
                extra, keep = waits[:-max_waits], waits[-max_waits:]
                if isinstance(inst, mybir.InstDMACopy):
                    for w in extra:
                        ab = copy.deepcopy(template_inst)
                        ab.name = nc.get_next_instruction_name()
                        ab.queue = inst.queue
                        ab.sync_info = mybir.SyncInfo(
                            on_wait=[w], on_update=[copy.deepcopy(junk)])
                        out.append(ab)
                else:
                    for w in extra:
                        out.append(mybir.InstNoOp(
                            name=nc.get_next_instruction_name(),
                            engine=inst.engine,
                            sync_info=mybir.SyncInfo(on_wait=[w], on_update=[])))
                inst.sync_info = mybir.SyncInfo(
                    on_wait=keep, on_update=list(si.on_update or []))
                out.append(inst)
            blk.instructions[:] = out


# ---------------------------------------------------------------------------
# Program


def _build_program(reps=None, use_mask=True):
    """Build the program.  reps=k wraps the whole computation in a
    hardware For loop executing it k times — used only for marginal
    HW timing (the ~100 ms axon dispatch swamps a single ~1 ms exec).
    use_mask=False elides the K=1 mask-starter matmuls (the mask input
    must then be all-ones so its additive row is exactly zero)."""
    import concourse.bass as bass
    import concourse.tile as tile
    from concourse import mybir
    from concourse.masks import make_identity

    dt = mybir.dt
    nc = bass.Bass("TRN2", target_bir_lowering=False, debug=False)

    x1T = nc.dram_tensor("x1T", [B_LOC, D, L], dt.float16, kind="ExternalInput").ap()
    x2T = nc.dram_tensor("x2T", [B_LOC, D, L], dt.float16, kind="ExternalInput").ap()
    WT = nc.dram_tensor("WT", [D, H], dt.float16, kind="ExternalInput").ap()
    W1T = nc.dram_tensor("W1T", [D, H], dt.float16, kind="ExternalInput").ap()
    x2n = nc.dram_tensor("x2n", [B_LOC, L, D], dt.bfloat16, kind="ExternalInput").ap()
    mrow = nc.dram_tensor("mrow", [B_LOC, 1, L], dt.bfloat16, kind="ExternalInput").ap()
    out = nc.dram_tensor("out", [B_LOC, L, D], dt.float32, kind="ExternalOutput").ap()

    with tile.TileContext(nc) as tc:
        with (
            tc.tile_pool(name="const", bufs=1) as cpool,
            tc.tile_pool(name="big", bufs=1) as bigp,
            tc.tile_pool(name="slab", bufs=2) as slabp,
            tc.tile_pool(name="r1pool", bufs=2) as r1p,
            tc.tile_pool(name="work", bufs=1) as workp,
            tc.tile_pool(name="tep", bufs=2) as tep,
            tc.tile_pool(name="small", bufs=2) as smallp,
            tc.tile_pool(name="outp", bufs=4) as outp,
            tc.tile_pool(name="ps_s", bufs=1, space="PSUM") as ps_s,
            tc.tile_pool(name="ps_p", bufs=2, space="PSUM") as ps_p,
            tc.tile_pool(name="ps_sm", bufs=2, space="PSUM") as ps_sm,
        ):
            tok = _make_wait_scratch(nc)

            identbf = cpool.tile([128, 128], dt.bfloat16, tag="identbf")
            make_identity(nc, identbf[:])
            ones_bf = cpool.tile([1, 128], dt.bfloat16, tag="ones_bf")
            nc.vector.memset(ones_bf[:], 1.0)

            # resident weights
            wt = bigp.tile([128, ND, H], dt.float16, tag="wt")
            w1t = bigp.tile([128, ND, H], dt.float16, tag="w1t")
            for dc in range(ND):
                nc.sync.dma_start(
                    wt[:, dc, :], WT.rearrange("(c p) h -> p c h", p=128)[:, dc, :])
            for dc in range(ND):
                nc.sync.dma_start(
                    w1t[:, dc, :], W1T.rearrange("(c p) h -> p c h", p=128)[:, dc, :])

            # per-batch resident tensors
            r2T = bigp.tile([128, NH, L], dt.float16, tag="r2T")
            t_x2n = bigp.tile([128, NJ3, D], dt.bfloat16, tag="t_x2n")
            t_m = workp.tile([1, L], dt.bfloat16, tag="t_m")
            sco = workp.tile([128, L], dt.float32, tag="sco")
            tET = workp.tile([128, NJ3, 128], dt.bfloat16, tag="tET")

            def emit_all_batches():
              pending = [None]
              for b in range(B_LOC):
                if pending[0] is not None:
                    pending[0]()
                    pending[0] = None
                # ---- batch loads ----
                for jc in range(NJ3):
                    nc.sync.dma_start(
                        t_x2n[:, jc, :],
                        x2n[b].rearrange("(c p) d -> p c d", p=128)[:, jc, :])
                nc.sync.dma_start(t_m[:], mrow[b])

                # ---- proj2: r2T = relu(W @ x2T) ----
                for s in range(NS):
                    xs = slabp.tile([128, ND, SW], dt.float16, tag="xslab")
                    for dc in range(ND):
                        nc.sync.dma_start(
                            xs[:, dc, :],
                            x2T[b].rearrange("(c p) l -> p c l", p=128)[
                                :, dc, s * SW:(s + 1) * SW])
                    for hc in range(NH):
                        psp = ps_p.tile([128, SW], dt.float32, tag="psp")
                        for dc in range(ND):
                            nc.tensor.matmul(
                                psp[:], wt[:, dc, hc * 128:(hc + 1) * 128],
                                xs[:, dc, :],
                                start=(dc == 0), stop=(dc == ND - 1))
                        nc.scalar.activation(
                            r2T[:, hc, s * SW:(s + 1) * SW], psp[:],
                            mybir.ActivationFunctionType.Relu)

                # ---- proj1 + attention, slab by slab ----
                for s in range(NS):
                    xs = slabp.tile([128, ND, SW], dt.float16, tag="xslab")
                    for dc in range(ND):
                        nc.sync.dma_start(
                            xs[:, dc, :],
                            x1T[b].rearrange("(c p) l -> p c l", p=128)[
                                :, dc, s * SW:(s + 1) * SW])
                    r1s = r1p.tile([128, NH, SW], dt.float16, tag="r1slab")
                    for hc in range(NH):
                        psp = ps_p.tile([128, SW], dt.float32, tag="psp")
                        for dc in range(ND):
                            nc.tensor.matmul(
                                psp[:], w1t[:, dc, hc * 128:(hc + 1) * 128],
                                xs[:, dc, :],
                                start=(dc == 0), stop=(dc == ND - 1))
                        nc.scalar.activation(
                            r1s[:, hc, :], psp[:],
                            mybir.ActivationFunctionType.Relu)

                    for il in range(IPS):
                        ic = s * IPS + il
                        isl = slice(il * 128, (il + 1) * 128)
                        pss = ps_s.tile([128, L], dt.float32, tag="pss")
                        if use_mask:
                            for jc in range(NJ):
                                nc.tensor.matmul(
                                    pss[:, jc * JW:(jc + 1) * JW], ones_bf[:],
                                    t_m[:, jc * JW:(jc + 1) * JW],
                                    start=True, stop=False,
                                    skip_group_check=True)
                        for hc in range(NH):
                            for jc in range(NJ):
                                nc.tensor.matmul(
                                    pss[:, jc * JW:(jc + 1) * JW],
                                    r1s[:, hc, isl],
                                    r2T[:, hc, jc * JW:(jc + 1) * JW],
                                    start=(not use_mask and hc == 0),
                                    stop=(hc == NH - 1),
                                    skip_group_check=True)
                        # PE post-work of the previous chunk goes here, so
                        # the PE never waits on this chunk's softmax.
                        if pending[0] is not None:
                            pending[0]()
                            pending[0] = None
                        # softmax chain (ACT/DVE)
                        nc.scalar.copy(sco[:], pss[:])
                        tneg = smallp.tile([128, 1], dt.float32, tag="tneg")
                        nc.vector.tensor_reduce(
                            tneg[:], sco[:], axis=mybir.AxisListType.X,
                            op=mybir.AluOpType.max, negate=True)
                        te = tep.tile([128, L], dt.bfloat16, tag="te")
                        tz = smallp.tile([128, 1], dt.float32, tag="tz")
                        nc.scalar.activation(
                            te[:], sco[:], mybir.ActivationFunctionType.Exp,
                            bias=tneg[:], scale=1.0, accum_out=tz[:])
                        tzi = smallp.tile([128, 1], dt.float32, tag="tzi")
                        nc.vector.reciprocal(tzi[:], tz[:])

                        def post(b=b, ic=ic, te=te, tzi=tzi):
                            for jc in range(NJ3):
                                pst = ps_sm.tile([128, 128], dt.bfloat16,
                                                 tag="psm")
                                nc.tensor.transpose(
                                    pst[:], te[:, jc * 128:(jc + 1) * 128],
                                    identbf[:])
                                nc.vector.tensor_copy(tET[:, jc, :], pst[:])
                            for dh in range(2):
                                pso = ps_sm.tile([128, 512], dt.float32,
                                                 tag="psm")
                                dsl = slice(dh * 512, (dh + 1) * 512)
                                for jc in range(NJ3):
                                    nc.tensor.matmul(
                                        pso[:], tET[:, jc, :],
                                        t_x2n[:, jc, dsl],
                                        start=(jc == 0), stop=(jc == NJ3 - 1))
                                tout = outp.tile([128, 512], dt.float32,
                                                 tag="tout")
                                nc.vector.tensor_scalar_mul(
                                    tout[:], pso[:], tzi[:])
                                nc.sync.dma_start(
                                    out[b, ic * 128:(ic + 1) * 128, dsl],
                                    tout[:])

                        pending[0] = post
              pending[0]()

            if reps:
                with tc.For_i(0, reps, 1):
                    emit_all_batches()
            else:
                emit_all_batches()

    _legalize_waits(nc, copy.deepcopy(tok))
    return nc


def _prepare_inputs(x1, x2, x2_mask, W, diagonal):
    import ml_dtypes
    x1 = np.ascontiguousarray(x1, dtype=np.float32)
    x2 = np.ascontiguousarray(x2, dtype=np.float32)
    W = np.ascontiguousarray(W, dtype=np.float32)
    diagonal = np.asarray(diagonal, dtype=np.float32)
    mask = np.asarray(x2_mask).astype(np.float32)

    assert np.all(diagonal > 0), "kernel fast path requires diagonal > 0"
    WT = np.ascontiguousarray(W.T, dtype=np.float32).astype(np.float16)
    if np.all(diagonal == 1.0):
        W1T = WT
    else:
        W1T = np.ascontiguousarray((W * diagonal[:, None]).T).astype(np.float16)

    x1T = np.ascontiguousarray(x1.transpose(0, 2, 1)).astype(np.float16)
    x2T = np.ascontiguousarray(x2.transpose(0, 2, 1)).astype(np.float16)
    x2nb = x2.astype(ml_dtypes.bfloat16)
    mrowv = ((1.0 - mask) * NEG)[:, None, :].astype(ml_dtypes.bfloat16)

    in_maps = []
    for c in range(NCORES):
        bs = slice(c * B_LOC, (c + 1) * B_LOC)
        in_maps.append({
            "x1T": x1T[bs],
            "x2T": x2T[bs],
            "WT": WT,
            "W1T": W1T,
            "x2n": x2nb[bs],
            "mrow": mrowv[bs],
        })
    return in_maps


_PROGS = {}


def _get_program(reps=None, use_mask=True):
    key = (reps, use_mask)
    if key not in _PROGS:
        _PROGS[key] = _build_program(reps=reps, use_mask=use_mask)
    return _PROGS[key]


def run(inputs, trace=False, **kw):
    """Run and return (output, BassKernelResults)."""
    from concourse.bass_utils import run_bass_kernel_spmd
    use_mask = not np.all(np.asarray(inputs["x2_mask"]) == 1)
    nc = _get_program(use_mask=use_mask)
    in_maps = _prepare_inputs(**inputs)
    try:
        res = run_bass_kernel_spmd(nc, in_maps, core_ids=list(range(NCORES)),
                                   trace=trace, **kw)
    except Exception:
        # first-compile hiccups have been observed under concurrent load;
        # the NEFF cache makes the retry cheap
        res = run_bass_kernel_spmd(nc, in_maps, core_ids=list(range(NCORES)),
                                   trace=trace, **kw)
    outs = [res.results[c]["out"] for c in range(NCORES)]
    full = np.concatenate(outs, axis=0).astype(np.float32)
    return full, res


def kernel(**inputs) -> np.ndarray:
    out, _ = run(inputs, trace=False)
    return out
